# revision 61
# baseline (speedup 1.0000x reference)
"""Trainium2 Bass kernel for nn_Model_39676907886571 (per-head attention, S=2048, d=3).

Math (per head h, fully head/data parallel, one head per NeuronCore):
  q_mat = query[h] @ x[h].T          (3, S)   -> q = q_mat viewed row-major as (S, 3)
  k_mat, v_mat likewise (the reshape is a memory-reinterpreting view, not a transpose)
  attn  = softmax(q @ k.T / sqrt(3)) (S, S)
  out   = (attn @ v).T               (3, S)

Device strategy (all on-chip, the S x S attention matrix never touches HBM):
  * qkv = W9 @ xT on the PE; a DRAM bounce reshapes the row-major flats into the
    "natural" (S, 3) triple layout that the weird view demands.
  * q^T / k^T are rebuilt by 16 PE transposes each, which produce a "u-order"
    permutation of the sequence axis (u = 128*c + p  <->  t_true = 16*p + c).
    Softmax sums over the key axis are permutation-invariant; the query axis is
    un-permuted at the end by a strided DVE write fused into the normalization.
  * E^T = exp(k-chunks^T @ q^T / sqrt(3)) keeps the key axis on partitions, so
    attn @ [1|v] needs no transposes and the softmax denominator falls out of
    the ones column of the [1|v] stationary operand.
  * Matmul operands are float32r (single-pass fp32 PE mode, 4x faster than the
    fp32hi/lo pair); PSUM ping-pong is managed manually so consecutive rounds
    only serialize through the exp (ACT is the bottleneck engine).
  * PSUM budget (8 banks): 2 x 3-bank ping-pong E^T tiles + 2 x 1-bank
    attn@[1|v] accumulators (per-s-chunk, accumulated across rounds in PSUM so
    no DVE op sits on the mm1 -> exp critical cycle).  The main loop is
    software-pipelined: mm1 of round g+1 issues before mm2 of round g so the
    in-order PE stream overlaps the exp; leftover q/k transpose groups ride
    idle PE slots mid-loop, writing into idle PSUM accumulators (dead until
    their first start=True matmul) so they never collide with live data.
    Cost model (TimelineSim): ~53 us/core, ACT-saturated (exp of the 2048^2
    attention matrix = 34 us floor at 128 lanes / 1.2 GHz).
"""

import numpy as np
from contextlib import ExitStack

import concourse.bass as bass
import concourse.tile as tile
from concourse import bacc, mybir
from concourse import bass_utils

F32 = mybir.dt.float32
F32R = mybir.dt.float32r

H, S, D = 8, 2048, 3
NCH = 16                # t-chunks of 128 (u-order blocks)
SQ = 512                # s-chunk width (one PSUM bank)
INV_SCALE = float(1.0 / np.sqrt(3.0))


def _r(ap):
    """Bitcast an fp32 AP to float32r (same bits)."""
    return ap.bitcast(F32R)


def build_program(reps=1):
    nc = bacc.Bacc("TRN2", num_devices=H, debug=False)
    xt_dram = nc.dram_tensor("xt", (3, S), F32, kind="ExternalInput")
    wt_dram = nc.dram_tensor("wt", (3, 9), F32, kind="ExternalInput")
    out_dram = nc.dram_tensor("out", (3, S), F32, kind="ExternalOutput")
    scratch = nc.dram_tensor("scratch", (3, 3 * S), F32, kind="Internal")

    with tile.TileContext(nc) as tc, ExitStack() as ctx:
        consts = ctx.enter_context(tc.tile_pool(name="consts", bufs=1))
        sb = ctx.enter_context(tc.tile_pool(name="sb", bufs=2 if reps > 1 else 1))
        es = ctx.enter_context(tc.tile_pool(name="es", bufs=4))
        ping = ctx.enter_context(tc.tile_pool(name="ping", bufs=1, space="PSUM"))
        accp = ctx.enter_context(tc.tile_pool(name="accp", bufs=1, space="PSUM"))

        # constants (shared across reps)
        ident_f = consts.tile([128, 128], F32)
        from concourse.masks import make_identity

        make_identity(nc, ident_f)
        ident = consts.tile([128, 128], F32R)
        nc.vector.tensor_copy(ident[:], ident_f[:])
        onesq_f = consts.tile([128, 16], F32)
        nc.vector.memset(onesq_f, 1.0)
        onesq = consts.tile([128, 16], F32R)
        nc.vector.tensor_copy(onesq[:], onesq_f[:])
        ones4 = consts.tile([1, 4], F32R)
        nc.vector.tensor_copy(ones4[:], onesq_f[0:1, 0:4])
        # prewarm the ACT exp table so the ~2.7us table load overlaps the prologue
        warm = consts.tile([1, 1], F32)
        nc.scalar.activation(warm[:], onesq_f[0:1, 0:1], mybir.ActivationFunctionType.Exp)

        for _rep in range(reps):
            _build_body(nc, tc, sb, es, ping, accp, ident, onesq, ones4,
                        xt_dram, wt_dram, out_dram, scratch)

    nc.compile()
    return nc


def _build_body(nc, tc, sb, es, ping, accp, ident, onesq, ones4,
                xt_dram, wt_dram, out_dram, scratch):
    psA = ping.tile([128, 3 * SQ], F32, tag="A")
    psB = ping.tile([128, 3 * SQ], F32, tag="B")
    pst_of = lambda g: psA if g % 2 == 0 else psB

    wT_sb = sb.tile([3, 9], F32R)
    nc.scalar.dma_start(wT_sb[:], _r(wt_dram.ap()))
    xT = sb.tile([3, S], F32R)
    nc.sync.dma_start(xT[:, 0 : 2 * SQ], _r(xt_dram.ap()[:, 0 : 2 * SQ]))
    nc.scalar.dma_start(xT[:, 2 * SQ : S], _r(xt_dram.ap()[:, 2 * SQ : S]))

    # qkv = W9 @ xT  (9, S), true t-order; PSUM -> SBUF -> DRAM bounce -> nats
    for m in range(4):
        tgt = psA[0:9, SQ * m : SQ * (m + 1)] if m < 3 else psB[0:9, 0:SQ]
        nc.tensor.matmul(
            tgt,
            lhsT=wT_sb[:],
            rhs=xT[:, SQ * m : SQ * (m + 1)],
            start=True,
            stop=True,
        )
    qkv_sb = sb.tile([9, S], F32)
    nc.scalar.copy(qkv_sb[:, 0 : 3 * SQ], psA[0:9, :])
    nc.vector.tensor_copy(qkv_sb[:, 3 * SQ : S], psB[0:9, 0:SQ])

    # warm the PE pstate during the otherwise idle DMA-bounce window so the
    # first transposes/matmuls run at full clock (writes are dead; mm1(0)
    # overwrites the same PSUM region later)
    for _w in range(10):
        nc.tensor.transpose(_r(psB[0:128, SQ : SQ + 128]), ident[:], ident[:])

    # natural (S, 3)-triple layout via a DRAM bounce (partition-crossing
    # reshape); per-tensor stores/loads pipelined across the two HWDGE queues
    nats = sb.tile([128, 144], F32R)
    scr = scratch.ap()
    nc.sync.dma_start(scr[0, :], qkv_sb[0:3, :])
    nc.scalar.dma_start(scr[1, :], qkv_sb[3:6, :])
    nc.scalar.dma_start(nats[:, 0:48], _r(scr[0, :]))
    nc.sync.dma_start(nats[:, 48:96], _r(scr[1, :]))
    # v's bounce is issued after the q/k loads: HWDGE descriptor generation is
    # a single serialized resource, and v is not needed until the first attn@v
    nc.sync.dma_start(scr[2, :], qkv_sb[6:9, :])
    nc.scalar.dma_start(nats[:, 96:144], _r(scr[2, :]))

    # vplus quads [1, v0, v1, v2] per chunk; built on the (otherwise idle) GPSIMD
    vplus = sb.tile([128, 64], F32R)
    nc.gpsimd.tensor_copy(vplus.rearrange("p (c q) -> p c q", q=4)[:, :, 0:1], onesq[:].unsqueeze(-1))
    for g in range(4):
        nc.gpsimd.tensor_copy(
            vplus.rearrange("p (c q) -> p c q", q=4)[:, 4 * g : 4 * (g + 1), 1:4],
            nats[:, 96 + 12 * g : 96 + 12 * (g + 1)].rearrange("p (c d) -> p c d", d=3),
        )

    # q^T / k^T in u-order via PE transposes of natural chunks.  Only the chunks
    # needed by round 0 are produced up front; the rest are interleaved into the
    # main loop's idle PE slots (writing to spare bank regions of the round's
    # PSUM tile after the exp has read it).
    qT_u = sb.tile([3, S], F32R)
    kT_u = sb.tile([3, S], F32R)

    def transpose_group_mm(src_off, grp, ps_region):
        for ci in range(4):
            c = 4 * grp + ci
            nc.tensor.transpose(
                _r(ps_region[0:3, 128 * ci : 128 * (ci + 1)]),
                nats[:, src_off + 3 * c : src_off + 3 * (c + 1)],
                ident[:],
            )

    def transpose_group_copy(dst, grp, ps_region, eng=None):
        if eng is None:
            nc.vector.tensor_copy(dst[:, SQ * grp : SQ * (grp + 1)], ps_region[0:3, :])
        else:
            eng.copy(dst[:, SQ * grp : SQ * (grp + 1)], ps_region[0:3, :])

    def transpose_group(dst, src_off, grp, ps_region, eng=None):
        transpose_group_mm(src_off, grp, ps_region)
        transpose_group_copy(dst, grp, ps_region, eng=eng)

    # ---------------- main attention loop (software-pipelined) ----------------
    # Rounds of <=3 t-chunks (the PSUM tiles are 3 banks); the attn@[1|v]
    # accumulation lives in its own 1-bank PSUM accumulator per s-chunk, so the
    # only cross-round serialization is mm1(next) -> exp: ACT runs back-to-back.
    # The first two rounds are 2 chunks wide: their exp leaves PSUM bank 2 free,
    # which hosts in-loop transposes without any wait on the exp.
    # acc rows: [denom, o0, o1, o2], cols in u-order of s.
    ROUND_CHUNKS = [(0, 1), (2, 3), (4, 5, 6), (7, 8, 9), (10, 11, 12), (13, 14, 15)]
    NR = len(ROUND_CHUNKS)
    recip = sb.tile([1, S], F32R)
    bc_sb = sb.tile([4, S], F32R)
    outv = sb.tile([4, S], F32)
    accs = [accp.tile([4, SQ], F32, tag=f"acc{j % 2}", name=f"acc_j{j}") for j in range(4)]

    def mm1(g):
        j, r = divmod(g, NR)
        pst = pst_of(g)
        for i, c in enumerate(ROUND_CHUNKS[r]):
            nc.tensor.matmul(
                pst[:, SQ * i : SQ * (i + 1)],
                lhsT=kT_u[:, 128 * c : 128 * (c + 1)],
                rhs=qT_u[:, SQ * j : SQ * (j + 1)],
                start=True,
                stop=True,
            )

    # remaining transpose groups ride the idle PE slots: PE work at round g
    # (into the free bank 2 on the 2-wide rounds, else into bank 0 after the
    # exp's read); the PSUM->SBUF copy early in round g+1 (it overlaps that
    # round's exp); the consuming mm1 issues one or more rounds later.
    # k3 uses the free bank of 2-wide round 0; the q-group for s-chunk j+1 is
    # transposed into accs[j+1] itself (idle until its first start=True mm2
    # overwrites it) -> no wait on any exp and no PSUM-bank conflict at all
    late_groups = {0: (kT_u, 48, 3), 4: (qT_u, 0, 1),
                   10: (qT_u, 0, 2), 16: (qT_u, 0, 3)}

    def late_region(g):
        return accs[g // NR + 1][0:4, :]

    def epilogue(j, bc_ps=None):
        # ---- per-s-chunk normalization, off the ACT critical path ----
        with nc.allow_low_precision(reason="float32r is 4-byte"):
            nc.vector.reciprocal(recip[:, SQ * j : SQ * (j + 1)], _r(accs[j][0:1, :]))
        if bc_ps is None:
            # mid-loop: broadcast on the idle GPSIMD
            bc = bc_sb[0:4, SQ * j : SQ * (j + 1)]
            nc.gpsimd.partition_broadcast(bc, recip[:, SQ * j : SQ * (j + 1)])
        else:
            # final chunk: PE is idle by now and its broadcast matmul is faster
            bc = bc_ps[0:4, :]
            nc.tensor.matmul(
                bc, lhsT=ones4[:], rhs=recip[:, SQ * j : SQ * (j + 1)],
                start=True, stop=True,
            )
        # normalization multiply fused with the u -> true-order un-permute of s:
        # outv[p, 16*pp + (4j+cc)] = acc[p, 128*cc + pp] * recip[...]
        nc.vector.tensor_mul(
            outv.rearrange("p (pp c) -> p pp c", c=NCH)[:, :, 4 * j : 4 * (j + 1)],
            accs[j][0:4, :].rearrange("p (c pp) -> p pp c", pp=128),
            bc.rearrange("p (c pp) -> p pp c", pp=128),
        )

    # prologue transpose groups: q0/k0 gate round 0; k1/k2/k3 run behind
    # mm1(0) on the in-order PE (they execute during the first exps), each in
    # a PSUM region whose next writer is late enough to hide the copy
    transpose_group(qT_u, 0, 0, psA[:, 0:SQ])
    # k0's copy rides the idle ACT so it runs in parallel with q0's DVE copy
    # instead of behind it (both gate mm1(0))
    transpose_group(kT_u, 48, 0, psB[:, 0:SQ], eng=nc.scalar)
    mm1(0)
    transpose_group(kT_u, 48, 1, psB[:, SQ : 2 * SQ])
    transpose_group(kT_u, 48, 2, psB[:, 2 * SQ : 3 * SQ])

    def mm2(g, e_t):
        j, r = divmod(g, NR)
        for i, c in enumerate(ROUND_CHUNKS[r]):
            nc.tensor.matmul(
                accs[j][0:4, :],
                lhsT=vplus[:, 4 * c : 4 * (c + 1)],
                rhs=e_t[:, SQ * i : SQ * (i + 1)],
                start=(r == 0 and i == 0),
                stop=(r == NR - 1 and i == len(ROUND_CHUNKS[r]) - 1),
            )

    # mm2 of round g is issued one round LATE (in body g+1): every matmul in
    # the PE stream then has its wait already satisfied at dispatch, so the
    # in-order PE never stalls between consecutive mm1 groups and the ACT
    # (exp) runs back-to-back even across short rounds and s-chunk boundaries.
    prev_e = None
    for g in range(4 * NR):
        j, r = divmod(g, NR)
        pst = pst_of(g)
        width = SQ * len(ROUND_CHUNKS[r])
        e_t = es.tile([128, 3 * SQ], F32R)
        nc.scalar.activation(
            e_t[:, 0:width], pst[:, 0:width],
            mybir.ActivationFunctionType.Exp, scale=INV_SCALE,
        )
        if g - 1 in late_groups:
            dst, off, grp = late_groups[g - 1]
            transpose_group_copy(dst, grp, late_region(g - 1))
        # next round's qk matmuls are independent of this exp: issue them first
        # so the in-order PE stream overlaps the exp (keeps ACT back-to-back)
        if g + 1 < 4 * NR:
            mm1(g + 1)
        if g >= 1:
            mm2(g - 1, prev_e)
        if r == 0 and j >= 1:
            epilogue(j - 1)
        if g in late_groups:
            dst, off, grp = late_groups[g]
            transpose_group_mm(off, grp, late_region(g))
        prev_e = e_t

    mm2(4 * NR - 1, prev_e)
    epilogue(3)
    nc.sync.dma_start(out_dram.ap(), outv[1:4, :])


_NC_CACHE = None


def _get_program():
    global _NC_CACHE
    if _NC_CACHE is None:
        _NC_CACHE = build_program()
    return _NC_CACHE


def kernel(x1, query, key_w, value, dropout_p=0):
    x1 = np.asarray(x1, dtype=np.float32)
    query = np.asarray(query, dtype=np.float32)
    key_w = np.asarray(key_w, dtype=np.float32)
    value = np.asarray(value, dtype=np.float32)

    in_maps = []
    for h in range(H):
        w9t = np.ascontiguousarray(
            np.concatenate([query[h], key_w[h], value[h]], axis=0).T
        )  # (3, 9)
        in_maps.append({"xt": np.ascontiguousarray(x1[h].T), "wt": w9t})

    # The axon terminal very occasionally drops a worker mid-execute
    # (NRT_EXEC_UNIT_UNRECOVERABLE); the kernel itself is deterministic, so
    # retry once with a freshly built program before giving up.
    global _NC_CACHE
    last_err = None
    for attempt in range(2):
        try:
            nc = _get_program()
            res = bass_utils.run_bass_kernel_spmd(nc, in_maps, core_ids=list(range(H)))
            return np.stack([res.results[h]["out"] for h in range(H)])
        except Exception as e:  # noqa: BLE001 - transient runtime faults only
            last_err = e
            _NC_CACHE = None
            import time as _time

            _time.sleep(5.0)
    raise last_err
